# revision 32
# baseline (speedup 1.0000x reference)
"""BertEmbeddings (7-way embedding sum + Time2Vec + LayerNorm) on 8 TRN2 cores.

Data-parallel: core i handles batch row i (2048 tokens); token t lives at
SBUF [partition t % 128, column t // 128]; 4 chunks of C=4 columns.

V3 design:
  - 4 batched full-S dma_gathers (2048 idx each): word+npi into one fp8 tile
    (paired for a DoubleRow matmul that sums both at half cost), combined
    modal+seg table (host-built 64-row fp8), posi bf16.
  - Time2Vec: DVE STT affine x = w/2pi * tau + (b/2pi + 4200); frac(x) is
    extracted EXACTLY as low mantissa bits (x in [4096,8192) has fixed
    exponent) with one 2x-mode bitwise_and; ACT Sin reads the integer
    directly (scale=-2pi/2048, bias=pi: sin(pi-2pi k/2048)=sin(2pi k/2048)).
  - v2t linear feature (col 767) via a tiny K=8 bf16 hi/lo matmul into PSUM.
  - PE sums: DoubleRow(word+npi) + cms fp8 + posi/sin_age/sin_del bf16
    identity streams into PSUM per token-column.
  - LayerNorm: bn_stats(512+256)/bn_aggr on PSUM, ACT normalize -> bf16 out.
"""

import math

import numpy as np

B, S, H = 8, 2048, 768
VOCAB, MODAL_V, SEG_V, NPI_V, MAX_POS = 32000, 16, 4, 10000, 2048
P = 128
COLS = S // P        # 16 token-columns
NCHUNK = 4
C = COLS // NCHUNK   # 4 columns per chunk
LN_EPS = 1e-12
INV_2PI = 1.0 / (2.0 * math.pi)
KSHIFT = 4200.0      # x + KSHIFT in [4096, 8192): fixed f32 exponent 2^12
H1 = H - 1

_cache = {}

import os as _os
_DBG_DR = _os.environ.get("K_DR", "1") == "1"        # DoubleRow pair stream
_DBG_V2T = _os.environ.get("K_V2T", "1") == "1"      # v2t K=128 matmul
_DBG_SIN = _os.environ.get("K_SIN", "1") == "1"      # t2v affine/and/sin path
_DBG_INPLACE = _os.environ.get("K_INPLACE", "1") == "1"  # in-place AND


def _build(use_gamma_beta: bool):
    import concourse.bacc as bacc
    import concourse.bass as bass
    import concourse.tile as tile
    from concourse import mybir
    from concourse.masks import make_identity
    from contextlib import ExitStack

    f32 = mybir.dt.float32
    bf16 = mybir.dt.bfloat16
    fp8 = mybir.dt.float8e4
    i32 = mybir.dt.int32
    i16 = mybir.dt.int16
    Alu = mybir.AluOpType
    Act = mybir.ActivationFunctionType

    nc = bacc.Bacc("TRN2", target_bir_lowering=False, debug=False,
                   dynamic_dma_scratch_size=40960, num_swdge_queues=2)

    d_widx = nc.dram_tensor("word_idx16", [P, P], i16, kind="ExternalInput")
    d_nidx = nc.dram_tensor("npi_idx16", [P, P], i16, kind="ExternalInput")
    d_cidx = nc.dram_tensor("cms_idx16", [P, P], i16, kind="ExternalInput")
    d_pidx = nc.dram_tensor("posi_idx16", [P, P], i16, kind="ExternalInput")
    d_age_tau = nc.dram_tensor("age_tau", [S], f32, kind="ExternalInput")
    d_delay_tau = nc.dram_tensor("delays_tau", [S], f32, kind="ExternalInput")
    d_word_tbl = nc.dram_tensor("word_table", [VOCAB, H], fp8, kind="ExternalInput")
    d_npi_tbl = nc.dram_tensor("npi_table", [NPI_V, H], fp8, kind="ExternalInput")
    d_cms_tbl = nc.dram_tensor("cms_table", [MODAL_V * SEG_V, H], fp8,
                               kind="ExternalInput")
    d_posi_tbl = nc.dram_tensor("posi_table", [MAX_POS, H], bf16, kind="ExternalInput")
    d_wa = nc.dram_tensor("wa", [H1], f32, kind="ExternalInput")  # w_age/2pi
    d_ba = nc.dram_tensor("ba", [H1], f32, kind="ExternalInput")  # b_age/2pi + K
    d_wd = nc.dram_tensor("wd", [H1], f32, kind="ExternalInput")
    d_bd = nc.dram_tensor("bd", [H1], f32, kind="ExternalInput")
    d_tv8 = nc.dram_tensor("tv8", [P, P], bf16, kind="ExternalInput")
    d_rhsv = nc.dram_tensor("rhsv", [P, COLS], bf16, kind="ExternalInput")
    if use_gamma_beta:
        d_gamma = nc.dram_tensor("ln_gamma", [H], bf16, kind="ExternalInput")
        d_beta = nc.dram_tensor("ln_beta", [H], bf16, kind="ExternalInput")
    d_out = nc.dram_tensor("out", [S, H], bf16, kind="ExternalOutput")

    def bcast_rows(handle, n, count, offset=0):
        ap = handle.ap()
        return bass.AP(tensor=ap.tensor, offset=offset, ap=[[0, n], [1, count]])

    with tile.TileContext(nc) as tc, ExitStack() as ctx:
        singles = ctx.enter_context(tc.tile_pool(name="singles", bufs=1))
        xwork = ctx.enter_context(tc.tile_pool(name="xwork", bufs=2))
        swork = ctx.enter_context(tc.tile_pool(name="swork", bufs=2))
        twork = ctx.enter_context(tc.tile_pool(name="twork", bufs=2))
        owork = ctx.enter_context(tc.tile_pool(name="owork", bufs=2))
        stwork = ctx.enter_context(tc.tile_pool(name="stwork", bufs=2))
        psum = ctx.enter_context(tc.tile_pool(name="psum", bufs=4, space="PSUM"))

        # ---- gathers: whole-S, batched (994ns fixed cost amortized) ----
        widx = singles.tile([P, P], i16)
        nidx = singles.tile([P, P], i16)
        cidx = singles.tile([P, P], i16)
        pidx = singles.tile([P, P], i16)
        nc.sync.dma_start(out=widx[:], in_=d_widx.ap())
        nc.sync.dma_start(out=nidx[:], in_=d_nidx.ap())
        nc.sync.dma_start(out=cidx[:], in_=d_cidx.ap())
        nc.sync.dma_start(out=pidx[:], in_=d_pidx.ap())

        # word at [:,0,:,:], npi at [:,1,:,:] -> DoubleRow pair dim
        g8 = singles.tile([P, 2, COLS, H], fp8)
        cms8 = singles.tile([P, COLS, H], fp8)
        posi16 = singles.tile([P, COLS, H], bf16)
        # chunk-ordered 512-idx gathers: chunk k's four tables land first so
        # chunk-0 compute starts after ~4 gathers instead of all 16
        GB = int(_os.environ.get("K_GB", "512"))  # idxs per gather
        NGRP = S // GB
        for g in range(NGRP):
            i0, i1 = g * GB // 16, (g + 1) * GB // 16
            c0_, c1_ = g * GB // P, (g + 1) * GB // P
            nc.gpsimd.dma_gather(
                out_ap=g8[:, 0, c0_:c1_, :], in_ap=d_word_tbl.ap(),
                idxs_ap=widx[:, i0:i1], num_idxs=GB, num_idxs_reg=GB,
                elem_size=H, queue_num=0)
            nc.gpsimd.dma_gather(
                out_ap=g8[:, 1, c0_:c1_, :], in_ap=d_npi_tbl.ap(),
                idxs_ap=nidx[:, i0:i1], num_idxs=GB, num_idxs_reg=GB,
                elem_size=H, queue_num=1)
            nc.gpsimd.dma_gather(
                out_ap=cms8[:, c0_:c1_, :], in_ap=d_cms_tbl.ap(),
                idxs_ap=cidx[:, i0:i1], num_idxs=GB, num_idxs_reg=GB,
                elem_size=H, queue_num=0)
            nc.gpsimd.dma_gather(
                out_ap=posi16[:, c0_:c1_, :], in_ap=d_posi_tbl.ap(),
                idxs_ap=pidx[:, i0:i1], num_idxs=GB, num_idxs_reg=GB,
                elem_size=H, queue_num=1)

        # ---- taus -> [P, 2*COLS] via PE transpose (cols 0..15 age, 16..31 del)
        identity16 = singles.tile([16, 16], f32)
        make_identity(nc, identity16[:])
        tau_raw = singles.tile([16, 2, P], f32)
        nc.sync.dma_start(out=tau_raw[:, 0, :],
                          in_=d_age_tau.ap().rearrange("(w p) -> w p", p=P))
        nc.sync.dma_start(out=tau_raw[:, 1, :],
                          in_=d_delay_tau.ap().rearrange("(w p) -> w p", p=P))
        tau_ps = psum.tile([P, 2 * COLS], f32, tag="mm")
        nc.tensor.transpose(out=tau_ps[:, 0:COLS], in_=tau_raw[:, 0, :],
                            identity=identity16[:])
        nc.tensor.transpose(out=tau_ps[:, COLS:2 * COLS], in_=tau_raw[:, 1, :],
                            identity=identity16[:])
        taus = singles.tile([P, 2 * COLS], f32)
        nc.vector.tensor_copy(out=taus[:], in_=tau_ps[:])

        # ---- identities + t2v params ----
        id8 = singles.tile([P, P], fp8)
        make_identity(nc, id8[:])
        id16 = singles.tile([P, P], bf16)
        make_identity(nc, id16[:])
        tv8 = singles.tile([P, P], bf16)
        nc.sync.dma_start(out=tv8[:], in_=d_tv8.ap())
        rhsv = singles.tile([P, COLS], bf16)
        nc.sync.dma_start(out=rhsv[:], in_=d_rhsv.ap())

        wa_t = singles.tile([P, H1], f32)
        ba_t = singles.tile([P, H1], f32)
        wd_t = singles.tile([P, H1], f32)
        bd_t = singles.tile([P, H1], f32)
        nc.sync.dma_start(out=wa_t[:], in_=bcast_rows(d_wa, P, H1))
        nc.sync.dma_start(out=ba_t[:], in_=bcast_rows(d_ba, P, H1))
        nc.sync.dma_start(out=wd_t[:], in_=bcast_rows(d_wd, P, H1))
        nc.sync.dma_start(out=bd_t[:], in_=bcast_rows(d_bd, P, H1))

        pi_t = singles.tile([P, 1], f32)
        nc.vector.memset(pi_t[:], math.pi)
        eps_t = singles.tile([P, 1], f32)
        nc.vector.memset(eps_t[:], LN_EPS)
        if use_gamma_beta:
            gamma_t = singles.tile([P, H], bf16)
            beta_t = singles.tile([P, H], bf16)
            nc.sync.dma_start(out=gamma_t[:], in_=bcast_rows(d_gamma, P, H))
            nc.sync.dma_start(out=beta_t[:], in_=bcast_rows(d_beta, P, H))

        out_v = d_out.ap().rearrange("(w p) h -> p w h", p=P)

        g8ap = g8[:]
        id8ap = id8[:]
        # lhsT pair view [K, 2, M]: same identity for both pair slots
        lhs_pair = bass.AP(tensor=id8ap.tensor, offset=id8ap.offset,
                           ap=[id8ap.ap[0], [0, 2], [1, P]])

        SIN_SCALE = -(2.0 * math.pi / 2048.0)

        for k in range(NCHUNK):
            w0c = k * C

            # ---- t2v affine: x = w'*tau + (b' + K), per (t2v, col) ----
            X = xwork.tile([P, C, 2, H1], f32)
            Sg = swork.tile([P, C, 2, H1], bf16)
            if _DBG_SIN:
                for t, (wt, bt) in enumerate(((wa_t, ba_t), (wd_t, bd_t))):
                    for c in range(C):
                        nc.vector.scalar_tensor_tensor(
                            out=X[:, c, t, :], in0=wt[:],
                            scalar=taus[:, t * COLS + w0c + c:
                                        t * COLS + w0c + c + 1],
                            in1=bt[:], op0=Alu.mult, op1=Alu.add,
                        )
                # frac bits: k_int = bits(x) & 0x7FF  (exact, 2^-11 grid)
                Xi = X[:, :, :, :].bitcast(i32)
                if _DBG_INPLACE:
                    nc.vector.tensor_scalar(out=Xi, in0=Xi, scalar1=0x7FF,
                                            scalar2=None, op0=Alu.bitwise_and)
                else:
                    X2 = xwork.tile([P, C, 2, H1], f32, tag="x2")
                    Xi2 = X2[:, :, :, :].bitcast(i32)
                    nc.vector.tensor_scalar(out=Xi2, in0=Xi, scalar1=0x7FF,
                                            scalar2=None, op0=Alu.bitwise_and)
                    Xi = Xi2
                # sin(pi - 2pi*k/2048) = sin(2pi*frac(x))
                nc.scalar.activation(out=Sg[:, :, :, :], in_=Xi, func=Act.Sin,
                                     bias=pi_t[:], scale=SIN_SCALE)
            else:
                nc.vector.memset(Sg[:, :, :, :], 0.0)
            # merge the two t2v sins on DVE (bf16 2x) -> one PE stream
            Tg = twork.tile([P, C, H1], bf16, tag="tg")
            nc.vector.tensor_tensor(out=Tg[:, :, :], in0=Sg[:, :, 0, :],
                                    in1=Sg[:, :, 1, :], op=Alu.add)

            # ---- per-column PE accumulation + stats (column pairs so the
            # psum pool [bufs=3] keeps pipelining) ----
            OUT = owork.tile([P, C, H], bf16)
            s0t = stwork.tile([P, C], f32, tag="s0")
            s2t = stwork.tile([P, C], f32, tag="s2")
            meant = stwork.tile([P, C], f32, tag="mean")
            msqt = stwork.tile([P, C], f32, tag="msq")
            vart = stwork.tile([P, C], f32, tag="var")
            rstd = stwork.tile([P, C], f32, tag="rstd")
            negmur = stwork.tile([P, C], f32, tag="negmur")
            for c0 in range(0, C, 2):
                pspair = []
                for c in (c0, c0 + 1):
                    j = w0c + c
                    ps = psum.tile([P, H], f32, tag="mm")
                    pspair.append(ps)
                    for n0, n1 in ((0, 512), (512, H)):
                        w = n1 - n0
                        # word+npi DoubleRow pair (starts the bank group)
                        rhs_pair = bass.AP(
                            tensor=g8ap.tensor,
                            offset=g8ap.offset + j * H + n0,
                            ap=[g8ap.ap[0], [COLS * H, 2], [1, w]],
                        )
                        nc.tensor.matmul(
                            out=ps[:, n0:n1], lhsT=lhs_pair,
                            rhs=rhs_pair, start=True, stop=False,
                            perf_mode=mybir.MatmulPerfMode.DoubleRow)
                        nc.tensor.matmul(out=ps[:, n0:n1], lhsT=id8ap,
                                         rhs=cms8[:, j, n0:n1], start=False,
                                         stop=False)
                        nc.tensor.matmul(out=ps[:, n0:n1], lhsT=id16[:],
                                         rhs=posi16[:, j, n0:n1], start=False,
                                         stop=False)
                    # v2t linear feature into col 767: K=128, rhs column j is
                    # zero except rows 8j..8j+7 (no data deps - run early)
                    nc.tensor.matmul(out=ps[:, H1:H], lhsT=tv8[:],
                                     rhs=rhsv[:, j:j + 1], start=False,
                                     stop=False)
                    # merged sin stream covers cols 0..766; closes both groups
                    for n0, n1 in ((0, 512), (512, H1)):
                        nc.tensor.matmul(out=ps[:, n0:n1], lhsT=id16[:],
                                         rhs=Tg[:, c, n0:n1],
                                         start=False, stop=True)

                    # stats on ACT: accum_out row-sums ride scratch passes
                    # (OUT[:, c, :] is overwritten by the final normalize)
                    nc.scalar.activation(out=OUT[:, c, :], in_=ps[:, :],
                                         func=Act.Square,
                                         accum_out=s2t[:, c:c + 1])
                    nc.scalar.activation(out=OUT[:, c, :], in_=ps[:, :],
                                         func=Act.Identity,
                                         accum_out=s0t[:, c:c + 1])

                # mean/var/rstd/negmur on [P, 2] slices
                sl = slice(c0, c0 + 2)
                nc.vector.tensor_scalar(out=meant[:, sl], in0=s0t[:, sl],
                                        scalar1=1.0 / H, scalar2=None,
                                        op0=Alu.mult)
                nc.vector.tensor_tensor(out=msqt[:, sl], in0=meant[:, sl],
                                        in1=meant[:, sl], op=Alu.mult)
                nc.vector.scalar_tensor_tensor(out=vart[:, sl],
                                               in0=s2t[:, sl],
                                               scalar=1.0 / H,
                                               in1=msqt[:, sl],
                                               op0=Alu.mult, op1=Alu.subtract)
                nc.scalar.activation(out=rstd[:, sl], in_=vart[:, sl],
                                     func=Act.Sqrt, bias=eps_t[:], scale=1.0)
                nc.vector.reciprocal(out=rstd[:, sl], in_=rstd[:, sl])
                nc.vector.scalar_tensor_tensor(out=negmur[:, sl],
                                               in0=meant[:, sl], scalar=-1.0,
                                               in1=rstd[:, sl],
                                               op0=Alu.mult, op1=Alu.mult)
                for ci, c in enumerate((c0, c0 + 1)):
                    nc.scalar.activation(out=OUT[:, c, :], in_=pspair[ci][:, :],
                                         func=Act.Identity,
                                         scale=rstd[:, c:c + 1],
                                         bias=negmur[:, c:c + 1])
            if use_gamma_beta:
                def mid_bcast(ap2, reps):
                    return bass.AP(tensor=ap2.tensor, offset=ap2.offset,
                                   ap=[ap2.ap[0], [0, reps], ap2.ap[1]])
                nc.vector.tensor_tensor(out=OUT[:, :, :], in0=OUT[:, :, :],
                                        in1=mid_bcast(gamma_t[:], C),
                                        op=Alu.mult)
                nc.vector.tensor_tensor(out=OUT[:, :, :], in0=OUT[:, :, :],
                                        in1=mid_bcast(beta_t[:], C),
                                        op=Alu.add)

            nc.sync.dma_start(out=out_v[:, w0c:w0c + C, :], in_=OUT[:, :, :])

    nc.compile()
    return nc


def _get_nc(use_gamma_beta: bool):
    key = ("nc", use_gamma_beta)
    if key not in _cache:
        _cache[key] = _build(use_gamma_beta)
    return _cache[key]


def _f32a(x):
    return np.ascontiguousarray(np.asarray(x), dtype=np.float32)


def _i32a(x):
    return np.ascontiguousarray(np.asarray(x), dtype=np.int32)


def _pack_idx16(ids_row):
    # ids_row [S] -> [128, 128] int16: idx i at [i % 16, i // 16], the 16-row
    # wrap replicated to the 8 gpsimd core groups.
    arr = np.zeros((16, P), dtype=np.int16)
    arr[:, :] = ids_row.reshape(P, 16).T.astype(np.int16)
    return np.tile(arr, (8, 1))


def _split_bf16(x):
    import ml_dtypes
    hi = x.astype(ml_dtypes.bfloat16)
    lo = (x - hi.astype(np.float32)).astype(ml_dtypes.bfloat16)
    return hi, lo


_last_use_gb = False


def _make_in_maps(inputs, use_gb):
    import ml_dtypes
    bf16np = ml_dtypes.bfloat16
    fp8np = ml_dtypes.float8_e4m3

    word_ids = _i32a(inputs["word_ids"]).reshape(B, S)
    modal_ids = _i32a(inputs["modalities_ids"]).reshape(B, S)
    seg_ids = _i32a(inputs["seg_ids"]).reshape(B, S)
    npi_ids = _i32a(inputs["npi_ids"]).reshape(B, S)
    posi_ids = _i32a(inputs["posi_ids"]).reshape(B, S)
    age_tau = _f32a(inputs["age_tau"]).reshape(B, S)
    delay_tau = _f32a(inputs["delays_tau"]).reshape(B, S)
    cms_ids = modal_ids + MODAL_V * seg_ids

    modal_tbl = _f32a(inputs["modalities_table"]).reshape(MODAL_V, H)
    seg_tbl = _f32a(inputs["seg_table"]).reshape(SEG_V, H)
    # cms row (m + 16*s) = modal[m] + seg[s]
    cms_tbl = (modal_tbl[:, None, :] + seg_tbl[None, :, :]).reshape(
        MODAL_V * SEG_V, H)

    w_age = _f32a(inputs["age_w"]).reshape(H1)
    b_age = _f32a(inputs["age_b"]).reshape(H1)
    w0_age = float(_f32a(inputs["age_w0"]).reshape(1)[0])
    b0_age = float(_f32a(inputs["age_b0"]).reshape(1)[0])
    w_del = _f32a(inputs["delay_w"]).reshape(H1)
    b_del = _f32a(inputs["delay_b"]).reshape(H1)
    w0_del = float(_f32a(inputs["delay_w0"]).reshape(1)[0])
    b0_del = float(_f32a(inputs["delay_b0"]).reshape(1)[0])

    b0s = np.float32(b0_age + b0_del)
    w0ah, w0al = _split_bf16(np.float32(w0_age).reshape(1))
    w0dh, w0dl = _split_bf16(np.float32(w0_del).reshape(1))
    b0h, b0l = _split_bf16(b0s.reshape(1))
    rhs8_vec = np.stack([w0ah[0], w0al[0], w0ah[0], w0dh[0], w0dl[0], w0dh[0],
                         b0h[0], b0l[0]])
    # rhsv[:, j] is zero except rows 8j..8j+7 = rhs8_vec (pairs with tv8 rows)
    rhsv = np.zeros((P, COLS), dtype=bf16np)
    for j in range(COLS):
        rhsv[8 * j:8 * j + 8, j] = rhs8_vec

    shared = {
        "word_table": np.ascontiguousarray(
            _f32a(inputs["word_table"]).reshape(VOCAB, H).astype(fp8np)),
        "npi_table": np.ascontiguousarray(
            _f32a(inputs["npi_table"]).reshape(NPI_V, H).astype(fp8np)),
        "cms_table": np.ascontiguousarray(cms_tbl.astype(fp8np)),
        "posi_table": np.ascontiguousarray(
            _f32a(inputs["posi_table"]).reshape(MAX_POS, H).astype(bf16np)),
        "wa": np.ascontiguousarray(w_age * INV_2PI),
        "ba": np.ascontiguousarray(b_age * INV_2PI + KSHIFT),
        "wd": np.ascontiguousarray(w_del * INV_2PI),
        "bd": np.ascontiguousarray(b_del * INV_2PI + KSHIFT),
        "rhsv": np.ascontiguousarray(rhsv),
    }
    if use_gb:
        shared["ln_gamma"] = np.ascontiguousarray(
            _f32a(inputs["ln_gamma"]).reshape(H).astype(bf16np))
        shared["ln_beta"] = np.ascontiguousarray(
            _f32a(inputs["ln_beta"]).reshape(H).astype(bf16np))

    in_maps = []
    for i in range(B):
        # tv8: per col j (8 rows): [tau_age hi, hi, lo, tau_del hi, hi, lo, 1, 1]
        ta = age_tau[i].reshape(COLS, P)     # [j, p]
        td = delay_tau[i].reshape(COLS, P)
        tah, tal = _split_bf16(ta)
        tdh, tdl = _split_bf16(td)
        tv8 = np.zeros((P, P), dtype=bf16np)
        ones = np.ones(P, dtype=bf16np)
        for j in range(COLS):
            tv8[8 * j + 0] = tah[j]
            tv8[8 * j + 1] = tah[j]
            tv8[8 * j + 2] = tal[j]
            tv8[8 * j + 3] = tdh[j]
            tv8[8 * j + 4] = tdh[j]
            tv8[8 * j + 5] = tdl[j]
            tv8[8 * j + 6] = ones
            tv8[8 * j + 7] = ones
        m = dict(shared)
        m.update(
            word_idx16=_pack_idx16(word_ids[i]),
            npi_idx16=_pack_idx16(npi_ids[i]),
            cms_idx16=_pack_idx16(cms_ids[i]),
            posi_idx16=_pack_idx16(posi_ids[i]),
            age_tau=age_tau[i],
            delays_tau=delay_tau[i],
            tv8=np.ascontiguousarray(tv8),
        )
        in_maps.append(m)
    return in_maps


def kernel(**inputs) -> np.ndarray:
    global _last_use_gb
    from concourse.bass_utils import run_bass_kernel_spmd

    gamma = _f32a(inputs["ln_gamma"])
    beta = _f32a(inputs["ln_beta"])
    use_gb = not (np.all(gamma == 1.0) and np.all(beta == 0.0))
    _last_use_gb = use_gb
    nc = _get_nc(use_gb)
    in_maps = _make_in_maps(inputs, use_gb)
    core_ids = list(range(B))
    res = run_bass_kernel_spmd(nc, in_maps, core_ids)
    out = np.stack(
        [np.asarray(res.results[i]["out"]).astype(np.float32) for i in core_ids],
        axis=0)
    return out


# revision 36
# speedup vs baseline: 1.1334x; 1.1334x over previous
"""BertEmbeddings (7-way embedding sum + Time2Vec + LayerNorm) on 8 TRN2 cores.

Data-parallel: core i handles batch row i (2048 tokens); token t lives at
SBUF [partition t % 128, column t // 128]; 4 chunks of C=4 columns.

V3 design:
  - 4 batched full-S dma_gathers (2048 idx each): word+npi into one fp8 tile
    (paired for a DoubleRow matmul that sums both at half cost), combined
    modal+seg table (host-built 64-row fp8), posi bf16.
  - Time2Vec: DVE STT affine x = w/2pi * tau + (b/2pi + 4200); frac(x) is
    extracted EXACTLY as low mantissa bits (x in [4096,8192) has fixed
    exponent) with one 2x-mode bitwise_and; ACT Sin reads the integer
    directly (scale=-2pi/2048, bias=pi: sin(pi-2pi k/2048)=sin(2pi k/2048)).
  - v2t linear feature (col 767) via a tiny K=8 bf16 hi/lo matmul into PSUM.
  - PE sums: DoubleRow(word+npi) + cms fp8 + posi/sin_age/sin_del bf16
    identity streams into PSUM per token-column.
  - LayerNorm: bn_stats(512+256)/bn_aggr on PSUM, ACT normalize -> bf16 out.
"""

import math

import numpy as np

B, S, H = 8, 2048, 768
VOCAB, MODAL_V, SEG_V, NPI_V, MAX_POS = 32000, 16, 4, 10000, 2048
P = 128
COLS = S // P        # 16 token-columns
NCHUNK = 4
C = COLS // NCHUNK   # 4 columns per chunk
LN_EPS = 1e-12
INV_2PI = 1.0 / (2.0 * math.pi)
KSHIFT = 4200.0      # x + KSHIFT in [4096, 8192): fixed f32 exponent 2^12
H1 = H - 1

_cache = {}

import os as _os
_DBG_DR = _os.environ.get("K_DR", "1") == "1"        # DoubleRow pair stream
_DBG_V2T = _os.environ.get("K_V2T", "1") == "1"      # v2t K=128 matmul
_DBG_SIN = _os.environ.get("K_SIN", "1") == "1"      # t2v affine/and/sin path
_DBG_INPLACE = _os.environ.get("K_INPLACE", "1") == "1"  # in-place AND


def _build(use_gamma_beta: bool):
    import concourse.bacc as bacc
    import concourse.bass as bass
    import concourse.tile as tile
    from concourse import mybir
    from concourse.masks import make_identity
    from contextlib import ExitStack

    f32 = mybir.dt.float32
    bf16 = mybir.dt.bfloat16
    fp8 = mybir.dt.float8e4
    i32 = mybir.dt.int32
    i16 = mybir.dt.int16
    Alu = mybir.AluOpType
    Act = mybir.ActivationFunctionType

    nc = bacc.Bacc("TRN2", target_bir_lowering=False, debug=False,
                   dynamic_dma_scratch_size=40960, num_swdge_queues=2)

    d_widx = nc.dram_tensor("word_idx16", [P, P], i16, kind="ExternalInput")
    d_nidx = nc.dram_tensor("npi_idx16", [P, P], i16, kind="ExternalInput")
    d_cidx = nc.dram_tensor("cms_idx16", [P, P], i16, kind="ExternalInput")
    d_pidx = nc.dram_tensor("posi_idx16", [P, P], i16, kind="ExternalInput")
    d_age_tau = nc.dram_tensor("age_tau", [S], f32, kind="ExternalInput")
    d_delay_tau = nc.dram_tensor("delays_tau", [S], f32, kind="ExternalInput")
    d_word_tbl = nc.dram_tensor("word_table", [VOCAB, H], fp8, kind="ExternalInput")
    d_npi_tbl = nc.dram_tensor("npi_table", [NPI_V, H], fp8, kind="ExternalInput")
    d_cms_tbl = nc.dram_tensor("cms_table", [MODAL_V * SEG_V, H], fp8,
                               kind="ExternalInput")
    d_posi_tbl = nc.dram_tensor("posi_table", [MAX_POS, H], bf16, kind="ExternalInput")
    d_wa = nc.dram_tensor("wa", [H1], f32, kind="ExternalInput")  # w_age/2pi
    d_ba = nc.dram_tensor("ba", [H1], f32, kind="ExternalInput")  # b_age/2pi + K
    d_wd = nc.dram_tensor("wd", [H1], f32, kind="ExternalInput")
    d_bd = nc.dram_tensor("bd", [H1], f32, kind="ExternalInput")
    d_tv8 = nc.dram_tensor("tv8", [P, P], bf16, kind="ExternalInput")
    d_rhsv = nc.dram_tensor("rhsv", [P, COLS], bf16, kind="ExternalInput")
    if use_gamma_beta:
        d_gamma = nc.dram_tensor("ln_gamma", [H], bf16, kind="ExternalInput")
        d_beta = nc.dram_tensor("ln_beta", [H], bf16, kind="ExternalInput")
    d_out = nc.dram_tensor("out", [S, H], bf16, kind="ExternalOutput")

    def bcast_rows(handle, n, count, offset=0):
        ap = handle.ap()
        return bass.AP(tensor=ap.tensor, offset=offset, ap=[[0, n], [1, count]])

    with tile.TileContext(nc) as tc, ExitStack() as ctx:
        singles = ctx.enter_context(tc.tile_pool(name="singles", bufs=1))
        xwork = ctx.enter_context(tc.tile_pool(name="xwork", bufs=2))
        swork = ctx.enter_context(tc.tile_pool(name="swork", bufs=2))
        twork = ctx.enter_context(tc.tile_pool(name="twork", bufs=2))
        owork = ctx.enter_context(tc.tile_pool(name="owork", bufs=2))
        stwork = ctx.enter_context(tc.tile_pool(name="stwork", bufs=2))
        psum = ctx.enter_context(tc.tile_pool(name="psum", bufs=4, space="PSUM"))

        # ---- t2v params first: they gate the DVE affine critical path ----
        tau_raw = singles.tile([16, 2, P], f32)
        nc.sync.dma_start(out=tau_raw[:, 0, :],
                          in_=d_age_tau.ap().rearrange("(w p) -> w p", p=P))
        nc.sync.dma_start(out=tau_raw[:, 1, :],
                          in_=d_delay_tau.ap().rearrange("(w p) -> w p", p=P))
        wa_t = singles.tile([P, H1], f32)
        ba_t = singles.tile([P, H1], f32)
        wd_t = singles.tile([P, H1], f32)
        bd_t = singles.tile([P, H1], f32)
        nc.sync.dma_start(out=wa_t[:], in_=bcast_rows(d_wa, P, H1))
        nc.sync.dma_start(out=ba_t[:], in_=bcast_rows(d_ba, P, H1))
        nc.sync.dma_start(out=wd_t[:], in_=bcast_rows(d_wd, P, H1))
        nc.sync.dma_start(out=bd_t[:], in_=bcast_rows(d_bd, P, H1))

        widx = singles.tile([P, P], i16)
        nidx = singles.tile([P, P], i16)
        cidx = singles.tile([P, P], i16)
        pidx = singles.tile([P, P], i16)
        nc.sync.dma_start(out=widx[:], in_=d_widx.ap())
        nc.sync.dma_start(out=nidx[:], in_=d_nidx.ap())
        nc.sync.dma_start(out=cidx[:], in_=d_cidx.ap())
        nc.sync.dma_start(out=pidx[:], in_=d_pidx.ap())

        # word at [:,0,:,:], npi at [:,1,:,:] -> DoubleRow pair dim
        g8 = singles.tile([P, 2, COLS, H], fp8)
        cms8 = singles.tile([P, COLS, H], fp8)
        posi16 = singles.tile([P, COLS, H], bf16)
        # chunk-ordered 512-idx gathers: chunk k's four tables land first so
        # chunk-0 compute starts after ~4 gathers instead of all 16
        GB = int(_os.environ.get("K_GB", "512"))  # idxs per gather
        NGRP = S // GB
        for g in range(NGRP):
            i0, i1 = g * GB // 16, (g + 1) * GB // 16
            c0_, c1_ = g * GB // P, (g + 1) * GB // P
            nc.gpsimd.dma_gather(
                out_ap=g8[:, 0, c0_:c1_, :], in_ap=d_word_tbl.ap(),
                idxs_ap=widx[:, i0:i1], num_idxs=GB, num_idxs_reg=GB,
                elem_size=H, queue_num=0)
            nc.gpsimd.dma_gather(
                out_ap=g8[:, 1, c0_:c1_, :], in_ap=d_npi_tbl.ap(),
                idxs_ap=nidx[:, i0:i1], num_idxs=GB, num_idxs_reg=GB,
                elem_size=H, queue_num=1)
            nc.gpsimd.dma_gather(
                out_ap=cms8[:, c0_:c1_, :], in_ap=d_cms_tbl.ap(),
                idxs_ap=cidx[:, i0:i1], num_idxs=GB, num_idxs_reg=GB,
                elem_size=H, queue_num=0)
            nc.gpsimd.dma_gather(
                out_ap=posi16[:, c0_:c1_, :], in_ap=d_posi_tbl.ap(),
                idxs_ap=pidx[:, i0:i1], num_idxs=GB, num_idxs_reg=GB,
                elem_size=H, queue_num=1)

        # ---- taus -> [P, 2*COLS] via PE transpose (cols 0..15 age, 16..31 del)
        identity16 = singles.tile([16, 16], f32)
        make_identity(nc, identity16[:])
        tau_ps = psum.tile([P, 2 * COLS], f32, tag="mm")
        nc.tensor.transpose(out=tau_ps[:, 0:COLS], in_=tau_raw[:, 0, :],
                            identity=identity16[:])
        nc.tensor.transpose(out=tau_ps[:, COLS:2 * COLS], in_=tau_raw[:, 1, :],
                            identity=identity16[:])
        taus = singles.tile([P, 2 * COLS], f32)
        nc.vector.tensor_copy(out=taus[:], in_=tau_ps[:])

        # ---- identities + t2v params ----
        id8 = singles.tile([P, P], fp8)
        make_identity(nc, id8[:])
        id16 = singles.tile([P, P], bf16)
        make_identity(nc, id16[:])
        tv8 = singles.tile([P, P], bf16)
        nc.sync.dma_start(out=tv8[:], in_=d_tv8.ap())
        rhsv = singles.tile([P, COLS], bf16)
        nc.sync.dma_start(out=rhsv[:], in_=d_rhsv.ap())


        pi_t = singles.tile([P, 1], f32)
        nc.vector.memset(pi_t[:], math.pi)
        eps_t = singles.tile([P, 1], f32)
        nc.vector.memset(eps_t[:], LN_EPS)
        if use_gamma_beta:
            gamma_t = singles.tile([P, H], bf16)
            beta_t = singles.tile([P, H], bf16)
            nc.sync.dma_start(out=gamma_t[:], in_=bcast_rows(d_gamma, P, H))
            nc.sync.dma_start(out=beta_t[:], in_=bcast_rows(d_beta, P, H))

        out_v = d_out.ap().rearrange("(w p) h -> p w h", p=P)

        g8ap = g8[:]
        id8ap = id8[:]
        # lhsT pair view [K, 2, M]: same identity for both pair slots
        lhs_pair = bass.AP(tensor=id8ap.tensor, offset=id8ap.offset,
                           ap=[id8ap.ap[0], [0, 2], [1, P]])

        SIN_SCALE = -(2.0 * math.pi / 2048.0)

        for k in range(NCHUNK):
            w0c = k * C

            # ---- t2v affine: x = w'*tau + (b' + K), per (t2v, col) ----
            X = xwork.tile([P, C, 2, H1], f32)
            Sg = swork.tile([P, C, 2, H1], bf16)
            if _DBG_SIN:
                for t, (wt, bt) in enumerate(((wa_t, ba_t), (wd_t, bd_t))):
                    for c in range(C):
                        nc.vector.scalar_tensor_tensor(
                            out=X[:, c, t, :], in0=wt[:],
                            scalar=taus[:, t * COLS + w0c + c:
                                        t * COLS + w0c + c + 1],
                            in1=bt[:], op0=Alu.mult, op1=Alu.add,
                        )
                # frac bits: k_int = bits(x) & 0x7FF  (exact, 2^-11 grid)
                Xi = X[:, :, :, :].bitcast(i32)
                if _DBG_INPLACE:
                    nc.vector.tensor_scalar(out=Xi, in0=Xi, scalar1=0x7FF,
                                            scalar2=None, op0=Alu.bitwise_and)
                else:
                    X2 = xwork.tile([P, C, 2, H1], f32, tag="x2")
                    Xi2 = X2[:, :, :, :].bitcast(i32)
                    nc.vector.tensor_scalar(out=Xi2, in0=Xi, scalar1=0x7FF,
                                            scalar2=None, op0=Alu.bitwise_and)
                    Xi = Xi2
                # sin(pi - 2pi*k/2048) = sin(2pi*frac(x))
                nc.scalar.activation(out=Sg[:, :, :, :], in_=Xi, func=Act.Sin,
                                     bias=pi_t[:], scale=SIN_SCALE)
            else:
                nc.vector.memset(Sg[:, :, :, :], 0.0)
            # merge the two t2v sins on DVE (bf16 2x) -> one PE stream
            Tg = twork.tile([P, C, H1], bf16, tag="tg")
            nc.vector.tensor_tensor(out=Tg[:, :, :], in0=Sg[:, :, 0, :],
                                    in1=Sg[:, :, 1, :], op=Alu.add)

            # ---- per-column PE accumulation + stats (column pairs so the
            # psum pool [bufs=3] keeps pipelining) ----
            OUT = owork.tile([P, C, H], bf16)
            st = stwork.tile([P, C, 2, 6], f32, tag="st")
            mv = stwork.tile([P, C, 2], f32, tag="mv")
            rstd = stwork.tile([P, C], f32, tag="rstd")
            negmur = stwork.tile([P, C], f32, tag="negmur")
            for c0 in range(0, C, 2):
                pspair = []
                for c in (c0, c0 + 1):
                    j = w0c + c
                    ps = psum.tile([P, H], f32, tag="mm")
                    pspair.append(ps)
                    for n0, n1 in ((0, 512), (512, H)):
                        w = n1 - n0
                        # word+npi DoubleRow pair (starts the bank group)
                        rhs_pair = bass.AP(
                            tensor=g8ap.tensor,
                            offset=g8ap.offset + j * H + n0,
                            ap=[g8ap.ap[0], [COLS * H, 2], [1, w]],
                        )
                        nc.tensor.matmul(
                            out=ps[:, n0:n1], lhsT=lhs_pair,
                            rhs=rhs_pair, start=True, stop=False,
                            perf_mode=mybir.MatmulPerfMode.DoubleRow)
                        nc.tensor.matmul(out=ps[:, n0:n1], lhsT=id8ap,
                                         rhs=cms8[:, j, n0:n1], start=False,
                                         stop=False)
                        nc.tensor.matmul(out=ps[:, n0:n1], lhsT=id16[:],
                                         rhs=posi16[:, j, n0:n1], start=False,
                                         stop=False)
                    # v2t linear feature into col 767: K=128, rhs column j is
                    # zero except rows 8j..8j+7 (no data deps - run early)
                    nc.tensor.matmul(out=ps[:, H1:H], lhsT=tv8[:],
                                     rhs=rhsv[:, j:j + 1], start=False,
                                     stop=False)
                    # merged sin stream covers cols 0..766; closes both groups
                    for n0, n1 in ((0, 512), (512, H1)):
                        nc.tensor.matmul(out=ps[:, n0:n1], lhsT=id16[:],
                                         rhs=Tg[:, c, n0:n1],
                                         start=False, stop=True)

                    nc.vector.bn_stats(out=st[:, c, 0, :], in_=ps[:, 0:384])
                    nc.vector.bn_stats(out=st[:, c, 1, :], in_=ps[:, 384:H])
                    nc.vector.bn_aggr(out=mv[:, c, :], in_=st[:, c, :, :])

                # rstd = 1/sqrt(var + eps); negmur = -mean * rstd
                mvap = mv[:]
                var_ap = bass.AP(tensor=mvap.tensor,
                                 offset=mvap.offset + 2 * c0 + 1,
                                 ap=[mvap.ap[0], [2, 2]])
                mean_ap = bass.AP(tensor=mvap.tensor,
                                  offset=mvap.offset + 2 * c0,
                                  ap=[mvap.ap[0], [2, 2]])
                nc.scalar.activation(out=rstd[:, c0:c0 + 2], in_=var_ap,
                                     func=Act.Sqrt, bias=eps_t[:], scale=1.0)
                nc.vector.reciprocal(out=rstd[:, c0:c0 + 2],
                                     in_=rstd[:, c0:c0 + 2])
                nc.vector.scalar_tensor_tensor(out=negmur[:, c0:c0 + 2],
                                               in0=mean_ap, scalar=-1.0,
                                               in1=rstd[:, c0:c0 + 2],
                                               op0=Alu.mult, op1=Alu.mult)
                for ci, c in enumerate((c0, c0 + 1)):
                    nc.scalar.activation(out=OUT[:, c, :], in_=pspair[ci][:, :],
                                         func=Act.Identity,
                                         scale=rstd[:, c:c + 1],
                                         bias=negmur[:, c:c + 1])
            if use_gamma_beta:
                def mid_bcast(ap2, reps):
                    return bass.AP(tensor=ap2.tensor, offset=ap2.offset,
                                   ap=[ap2.ap[0], [0, reps], ap2.ap[1]])
                nc.vector.tensor_tensor(out=OUT[:, :, :], in0=OUT[:, :, :],
                                        in1=mid_bcast(gamma_t[:], C),
                                        op=Alu.mult)
                nc.vector.tensor_tensor(out=OUT[:, :, :], in0=OUT[:, :, :],
                                        in1=mid_bcast(beta_t[:], C),
                                        op=Alu.add)

            nc.sync.dma_start(out=out_v[:, w0c:w0c + C, :], in_=OUT[:, :, :])

    nc.compile()
    return nc


def _get_nc(use_gamma_beta: bool):
    key = ("nc", use_gamma_beta)
    if key not in _cache:
        _cache[key] = _build(use_gamma_beta)
    return _cache[key]


def _f32a(x):
    return np.ascontiguousarray(np.asarray(x), dtype=np.float32)


def _i32a(x):
    return np.ascontiguousarray(np.asarray(x), dtype=np.int32)


def _pack_idx16(ids_row):
    # ids_row [S] -> [128, 128] int16: idx i at [i % 16, i // 16], the 16-row
    # wrap replicated to the 8 gpsimd core groups.
    arr = np.zeros((16, P), dtype=np.int16)
    arr[:, :] = ids_row.reshape(P, 16).T.astype(np.int16)
    return np.tile(arr, (8, 1))


def _split_bf16(x):
    import ml_dtypes
    hi = x.astype(ml_dtypes.bfloat16)
    lo = (x - hi.astype(np.float32)).astype(ml_dtypes.bfloat16)
    return hi, lo


_last_use_gb = False


def _make_in_maps(inputs, use_gb):
    import ml_dtypes
    bf16np = ml_dtypes.bfloat16
    fp8np = ml_dtypes.float8_e4m3

    word_ids = _i32a(inputs["word_ids"]).reshape(B, S)
    modal_ids = _i32a(inputs["modalities_ids"]).reshape(B, S)
    seg_ids = _i32a(inputs["seg_ids"]).reshape(B, S)
    npi_ids = _i32a(inputs["npi_ids"]).reshape(B, S)
    posi_ids = _i32a(inputs["posi_ids"]).reshape(B, S)
    age_tau = _f32a(inputs["age_tau"]).reshape(B, S)
    delay_tau = _f32a(inputs["delays_tau"]).reshape(B, S)
    cms_ids = modal_ids + MODAL_V * seg_ids

    modal_tbl = _f32a(inputs["modalities_table"]).reshape(MODAL_V, H)
    seg_tbl = _f32a(inputs["seg_table"]).reshape(SEG_V, H)
    # cms row (m + 16*s) = modal[m] + seg[s]
    cms_tbl = (modal_tbl[:, None, :] + seg_tbl[None, :, :]).reshape(
        MODAL_V * SEG_V, H)

    w_age = _f32a(inputs["age_w"]).reshape(H1)
    b_age = _f32a(inputs["age_b"]).reshape(H1)
    w0_age = float(_f32a(inputs["age_w0"]).reshape(1)[0])
    b0_age = float(_f32a(inputs["age_b0"]).reshape(1)[0])
    w_del = _f32a(inputs["delay_w"]).reshape(H1)
    b_del = _f32a(inputs["delay_b"]).reshape(H1)
    w0_del = float(_f32a(inputs["delay_w0"]).reshape(1)[0])
    b0_del = float(_f32a(inputs["delay_b0"]).reshape(1)[0])

    b0s = np.float32(b0_age + b0_del)
    w0ah, w0al = _split_bf16(np.float32(w0_age).reshape(1))
    w0dh, w0dl = _split_bf16(np.float32(w0_del).reshape(1))
    b0h, b0l = _split_bf16(b0s.reshape(1))
    rhs8_vec = np.stack([w0ah[0], w0al[0], w0ah[0], w0dh[0], w0dl[0], w0dh[0],
                         b0h[0], b0l[0]])
    # rhsv[:, j] is zero except rows 8j..8j+7 = rhs8_vec (pairs with tv8 rows)
    rhsv = np.zeros((P, COLS), dtype=bf16np)
    for j in range(COLS):
        rhsv[8 * j:8 * j + 8, j] = rhs8_vec

    shared = {
        "word_table": np.ascontiguousarray(
            _f32a(inputs["word_table"]).reshape(VOCAB, H).astype(fp8np)),
        "npi_table": np.ascontiguousarray(
            _f32a(inputs["npi_table"]).reshape(NPI_V, H).astype(fp8np)),
        "cms_table": np.ascontiguousarray(cms_tbl.astype(fp8np)),
        "posi_table": np.ascontiguousarray(
            _f32a(inputs["posi_table"]).reshape(MAX_POS, H).astype(bf16np)),
        "wa": np.ascontiguousarray(w_age * INV_2PI),
        "ba": np.ascontiguousarray(b_age * INV_2PI + KSHIFT),
        "wd": np.ascontiguousarray(w_del * INV_2PI),
        "bd": np.ascontiguousarray(b_del * INV_2PI + KSHIFT),
        "rhsv": np.ascontiguousarray(rhsv),
    }
    if use_gb:
        shared["ln_gamma"] = np.ascontiguousarray(
            _f32a(inputs["ln_gamma"]).reshape(H).astype(bf16np))
        shared["ln_beta"] = np.ascontiguousarray(
            _f32a(inputs["ln_beta"]).reshape(H).astype(bf16np))

    in_maps = []
    for i in range(B):
        # tv8: per col j (8 rows): [tau_age hi, hi, lo, tau_del hi, hi, lo, 1, 1]
        ta = age_tau[i].reshape(COLS, P)     # [j, p]
        td = delay_tau[i].reshape(COLS, P)
        tah, tal = _split_bf16(ta)
        tdh, tdl = _split_bf16(td)
        tv8 = np.zeros((P, P), dtype=bf16np)
        ones = np.ones(P, dtype=bf16np)
        for j in range(COLS):
            tv8[8 * j + 0] = tah[j]
            tv8[8 * j + 1] = tah[j]
            tv8[8 * j + 2] = tal[j]
            tv8[8 * j + 3] = tdh[j]
            tv8[8 * j + 4] = tdh[j]
            tv8[8 * j + 5] = tdl[j]
            tv8[8 * j + 6] = ones
            tv8[8 * j + 7] = ones
        m = dict(shared)
        m.update(
            word_idx16=_pack_idx16(word_ids[i]),
            npi_idx16=_pack_idx16(npi_ids[i]),
            cms_idx16=_pack_idx16(cms_ids[i]),
            posi_idx16=_pack_idx16(posi_ids[i]),
            age_tau=age_tau[i],
            delays_tau=delay_tau[i],
            tv8=np.ascontiguousarray(tv8),
        )
        in_maps.append(m)
    return in_maps


def kernel(**inputs) -> np.ndarray:
    global _last_use_gb
    from concourse.bass_utils import run_bass_kernel_spmd

    gamma = _f32a(inputs["ln_gamma"])
    beta = _f32a(inputs["ln_beta"])
    use_gb = not (np.all(gamma == 1.0) and np.all(beta == 0.0))
    _last_use_gb = use_gb
    nc = _get_nc(use_gb)
    in_maps = _make_in_maps(inputs, use_gb)
    core_ids = list(range(B))
    res = run_bass_kernel_spmd(nc, in_maps, core_ids)
    out = np.stack(
        [np.asarray(res.results[i]["out"]).astype(np.float32) for i in core_ids],
        axis=0)
    return out


# revision 37
# speedup vs baseline: 1.1951x; 1.0545x over previous
"""BertEmbeddings (7-way embedding sum + Time2Vec + LayerNorm) on 8 TRN2 cores.

Data-parallel: core i handles batch row i (2048 tokens); token t lives at
SBUF [partition t % 128, column t // 128]; 4 chunks of C=4 columns.

V3 design:
  - 4 batched full-S dma_gathers (2048 idx each): word+npi into one fp8 tile
    (paired for a DoubleRow matmul that sums both at half cost), combined
    modal+seg table (host-built 64-row fp8), posi bf16.
  - Time2Vec: DVE STT affine x = w/2pi * tau + (b/2pi + 4200); frac(x) is
    extracted EXACTLY as low mantissa bits (x in [4096,8192) has fixed
    exponent) with one 2x-mode bitwise_and; ACT Sin reads the integer
    directly (scale=-2pi/2048, bias=pi: sin(pi-2pi k/2048)=sin(2pi k/2048)).
  - v2t linear feature (col 767) via a tiny K=8 bf16 hi/lo matmul into PSUM.
  - PE sums: DoubleRow(word+npi) + cms fp8 + posi/sin_age/sin_del bf16
    identity streams into PSUM per token-column.
  - LayerNorm: bn_stats(512+256)/bn_aggr on PSUM, ACT normalize -> bf16 out.
"""

import math

import numpy as np

B, S, H = 8, 2048, 768
VOCAB, MODAL_V, SEG_V, NPI_V, MAX_POS = 32000, 16, 4, 10000, 2048
P = 128
COLS = S // P        # 16 token-columns
NCHUNK = 4
C = COLS // NCHUNK   # 4 columns per chunk
LN_EPS = 1e-12
INV_2PI = 1.0 / (2.0 * math.pi)
KSHIFT = 4200.0      # x + KSHIFT in [4096, 8192): fixed f32 exponent 2^12
H1 = H - 1

_cache = {}

import os as _os
_DBG_DR = _os.environ.get("K_DR", "1") == "1"        # DoubleRow pair stream
_DBG_V2T = _os.environ.get("K_V2T", "1") == "1"      # v2t K=128 matmul
_DBG_SIN = _os.environ.get("K_SIN", "1") == "1"      # t2v affine/and/sin path
_DBG_INPLACE = _os.environ.get("K_INPLACE", "1") == "1"  # in-place AND


def _build(use_gamma_beta: bool):
    import concourse.bacc as bacc
    import concourse.bass as bass
    import concourse.tile as tile
    from concourse import mybir
    from concourse.masks import make_identity
    from contextlib import ExitStack

    f32 = mybir.dt.float32
    bf16 = mybir.dt.bfloat16
    fp8 = mybir.dt.float8e4
    i32 = mybir.dt.int32
    i16 = mybir.dt.int16
    Alu = mybir.AluOpType
    Act = mybir.ActivationFunctionType

    nc = bacc.Bacc("TRN2", target_bir_lowering=False, debug=False,
                   dynamic_dma_scratch_size=40960, num_swdge_queues=2)

    d_widx = nc.dram_tensor("word_idx16", [P, P], i16, kind="ExternalInput")
    d_nidx = nc.dram_tensor("npi_idx16", [P, P], i16, kind="ExternalInput")
    d_cidx = nc.dram_tensor("cms_idx16", [P, P], i16, kind="ExternalInput")
    d_pidx = nc.dram_tensor("posi_idx16", [P, P], i16, kind="ExternalInput")
    d_age_tau = nc.dram_tensor("age_tau", [S], f32, kind="ExternalInput")
    d_delay_tau = nc.dram_tensor("delays_tau", [S], f32, kind="ExternalInput")
    d_word_tbl = nc.dram_tensor("word_table", [VOCAB, H], fp8, kind="ExternalInput")
    d_npi_tbl = nc.dram_tensor("npi_table", [NPI_V, H], fp8, kind="ExternalInput")
    d_cms_tbl = nc.dram_tensor("cms_table", [MODAL_V * SEG_V, H], fp8,
                               kind="ExternalInput")
    d_posi_tbl = nc.dram_tensor("posi_table", [MAX_POS, H], bf16, kind="ExternalInput")
    d_wa = nc.dram_tensor("wa", [H1], f32, kind="ExternalInput")  # w_age/2pi
    d_ba = nc.dram_tensor("ba", [H1], f32, kind="ExternalInput")  # b_age/2pi + K
    d_wd = nc.dram_tensor("wd", [H1], f32, kind="ExternalInput")
    d_bd = nc.dram_tensor("bd", [H1], f32, kind="ExternalInput")
    d_tv8 = nc.dram_tensor("tv8", [P, P], bf16, kind="ExternalInput")
    d_rhsv = nc.dram_tensor("rhsv", [P, COLS], bf16, kind="ExternalInput")
    if use_gamma_beta:
        d_gamma = nc.dram_tensor("ln_gamma", [H], bf16, kind="ExternalInput")
        d_beta = nc.dram_tensor("ln_beta", [H], bf16, kind="ExternalInput")
    d_out = nc.dram_tensor("out", [S, H], bf16, kind="ExternalOutput")

    def bcast_rows(handle, n, count, offset=0):
        ap = handle.ap()
        return bass.AP(tensor=ap.tensor, offset=offset, ap=[[0, n], [1, count]])

    with tile.TileContext(nc) as tc, ExitStack() as ctx:
        singles = ctx.enter_context(tc.tile_pool(name="singles", bufs=1))
        xwork = ctx.enter_context(tc.tile_pool(name="xwork", bufs=2))
        swork = ctx.enter_context(tc.tile_pool(name="swork", bufs=2))
        twork = ctx.enter_context(tc.tile_pool(name="twork", bufs=2))
        owork = ctx.enter_context(tc.tile_pool(name="owork", bufs=2))
        stwork = ctx.enter_context(tc.tile_pool(name="stwork", bufs=2))
        psum = ctx.enter_context(tc.tile_pool(name="psum", bufs=4, space="PSUM"))

        # ---- gathers: whole-S, batched (994ns fixed cost amortized) ----
        widx = singles.tile([P, P], i16)
        nidx = singles.tile([P, P], i16)
        cidx = singles.tile([P, P], i16)
        pidx = singles.tile([P, P], i16)
        nc.sync.dma_start(out=widx[:], in_=d_widx.ap())
        nc.sync.dma_start(out=nidx[:], in_=d_nidx.ap())
        nc.sync.dma_start(out=cidx[:], in_=d_cidx.ap())
        nc.sync.dma_start(out=pidx[:], in_=d_pidx.ap())

        # word at [:,0,:,:], npi at [:,1,:,:] -> DoubleRow pair dim
        g8 = singles.tile([P, 2, COLS, H], fp8)
        cms8 = singles.tile([P, COLS, H], fp8)
        posi16 = singles.tile([P, COLS, H], bf16)
        # chunk-ordered 512-idx gathers: chunk k's four tables land first so
        # chunk-0 compute starts after ~4 gathers instead of all 16
        GB = int(_os.environ.get("K_GB", "512"))  # idxs per gather
        NGRP = S // GB
        for g in range(NGRP):
            i0, i1 = g * GB // 16, (g + 1) * GB // 16
            c0_, c1_ = g * GB // P, (g + 1) * GB // P
            nc.gpsimd.dma_gather(
                out_ap=g8[:, 0, c0_:c1_, :], in_ap=d_word_tbl.ap(),
                idxs_ap=widx[:, i0:i1], num_idxs=GB, num_idxs_reg=GB,
                elem_size=H, queue_num=0)
            nc.gpsimd.dma_gather(
                out_ap=g8[:, 1, c0_:c1_, :], in_ap=d_npi_tbl.ap(),
                idxs_ap=nidx[:, i0:i1], num_idxs=GB, num_idxs_reg=GB,
                elem_size=H, queue_num=1)
            nc.gpsimd.dma_gather(
                out_ap=cms8[:, c0_:c1_, :], in_ap=d_cms_tbl.ap(),
                idxs_ap=cidx[:, i0:i1], num_idxs=GB, num_idxs_reg=GB,
                elem_size=H, queue_num=0)
            nc.gpsimd.dma_gather(
                out_ap=posi16[:, c0_:c1_, :], in_ap=d_posi_tbl.ap(),
                idxs_ap=pidx[:, i0:i1], num_idxs=GB, num_idxs_reg=GB,
                elem_size=H, queue_num=1)

        # ---- taus -> [P, 2*COLS] via PE transpose (cols 0..15 age, 16..31 del)
        identity16 = singles.tile([16, 16], f32)
        make_identity(nc, identity16[:])
        tau_raw = singles.tile([16, 2, P], f32)
        nc.sync.dma_start(out=tau_raw[:, 0, :],
                          in_=d_age_tau.ap().rearrange("(w p) -> w p", p=P))
        nc.sync.dma_start(out=tau_raw[:, 1, :],
                          in_=d_delay_tau.ap().rearrange("(w p) -> w p", p=P))
        tau_ps = psum.tile([P, 2 * COLS], f32, tag="mm")
        nc.tensor.transpose(out=tau_ps[:, 0:COLS], in_=tau_raw[:, 0, :],
                            identity=identity16[:])
        nc.tensor.transpose(out=tau_ps[:, COLS:2 * COLS], in_=tau_raw[:, 1, :],
                            identity=identity16[:])
        taus = singles.tile([P, 2 * COLS], f32)
        nc.vector.tensor_copy(out=taus[:], in_=tau_ps[:])

        # ---- identities + t2v params ----
        id8 = singles.tile([P, P], fp8)
        make_identity(nc, id8[:])
        id16 = singles.tile([P, P], bf16)
        make_identity(nc, id16[:])
        tv8 = singles.tile([P, P], bf16)
        nc.sync.dma_start(out=tv8[:], in_=d_tv8.ap())
        rhsv = singles.tile([P, COLS], bf16)
        nc.sync.dma_start(out=rhsv[:], in_=d_rhsv.ap())


        wa_t = singles.tile([P, H1], f32)
        ba_t = singles.tile([P, H1], f32)
        wd_t = singles.tile([P, H1], f32)
        bd_t = singles.tile([P, H1], f32)
        nc.sync.dma_start(out=wa_t[:], in_=bcast_rows(d_wa, P, H1))
        nc.sync.dma_start(out=ba_t[:], in_=bcast_rows(d_ba, P, H1))
        nc.sync.dma_start(out=wd_t[:], in_=bcast_rows(d_wd, P, H1))
        nc.sync.dma_start(out=bd_t[:], in_=bcast_rows(d_bd, P, H1))

        pi_t = singles.tile([P, 1], f32)
        nc.vector.memset(pi_t[:], math.pi)
        eps_t = singles.tile([P, 1], f32)
        nc.vector.memset(eps_t[:], LN_EPS)
        if use_gamma_beta:
            gamma_t = singles.tile([P, H], bf16)
            beta_t = singles.tile([P, H], bf16)
            nc.sync.dma_start(out=gamma_t[:], in_=bcast_rows(d_gamma, P, H))
            nc.sync.dma_start(out=beta_t[:], in_=bcast_rows(d_beta, P, H))

        out_v = d_out.ap().rearrange("(w p) h -> p w h", p=P)

        g8ap = g8[:]
        id8ap = id8[:]
        # lhsT pair view [K, 2, M]: same identity for both pair slots
        lhs_pair = bass.AP(tensor=id8ap.tensor, offset=id8ap.offset,
                           ap=[id8ap.ap[0], [0, 2], [1, P]])

        SIN_SCALE = -(2.0 * math.pi / 2048.0)

        for k in range(NCHUNK):
            w0c = k * C

            # ---- t2v affine: x = w'*tau + (b' + K), per (t2v, col) ----
            X = xwork.tile([P, C, 2, H1], f32)
            Sg = swork.tile([P, C, 2, H1], bf16)
            if _DBG_SIN:
                for t, (wt, bt) in enumerate(((wa_t, ba_t), (wd_t, bd_t))):
                    for c in range(C):
                        nc.vector.scalar_tensor_tensor(
                            out=X[:, c, t, :], in0=wt[:],
                            scalar=taus[:, t * COLS + w0c + c:
                                        t * COLS + w0c + c + 1],
                            in1=bt[:], op0=Alu.mult, op1=Alu.add,
                        )
                # frac bits: k_int = bits(x) & 0x7FF  (exact, 2^-11 grid)
                Xi = X[:, :, :, :].bitcast(i32)
                if _DBG_INPLACE:
                    nc.vector.tensor_scalar(out=Xi, in0=Xi, scalar1=0x7FF,
                                            scalar2=None, op0=Alu.bitwise_and)
                else:
                    X2 = xwork.tile([P, C, 2, H1], f32, tag="x2")
                    Xi2 = X2[:, :, :, :].bitcast(i32)
                    nc.vector.tensor_scalar(out=Xi2, in0=Xi, scalar1=0x7FF,
                                            scalar2=None, op0=Alu.bitwise_and)
                    Xi = Xi2
                # sin(pi - 2pi*k/2048) = sin(2pi*frac(x))
                nc.scalar.activation(out=Sg[:, :, :, :], in_=Xi, func=Act.Sin,
                                     bias=pi_t[:], scale=SIN_SCALE)
            else:
                nc.vector.memset(Sg[:, :, :, :], 0.0)
            # merge the two t2v sins on DVE (bf16 2x) -> one PE stream
            Tg = twork.tile([P, C, H1], bf16, tag="tg")
            nc.vector.tensor_tensor(out=Tg[:, :, :], in0=Sg[:, :, 0, :],
                                    in1=Sg[:, :, 1, :], op=Alu.add)

            # ---- per-column PE accumulation + stats (column pairs so the
            # psum pool [bufs=3] keeps pipelining) ----
            OUT = owork.tile([P, C, H], bf16)
            st = stwork.tile([P, C, 2, 6], f32, tag="st")
            mv = stwork.tile([P, C, 2], f32, tag="mv")
            rstd = stwork.tile([P, C], f32, tag="rstd")
            negmur = stwork.tile([P, C], f32, tag="negmur")
            for c0 in range(0, C, 2):
                pspair = []
                for c in (c0, c0 + 1):
                    j = w0c + c
                    ps = psum.tile([P, H], f32, tag="mm")
                    pspair.append(ps)
                    for n0, n1 in ((0, 512), (512, H)):
                        w = n1 - n0
                        # word+npi DoubleRow pair (starts the bank group)
                        rhs_pair = bass.AP(
                            tensor=g8ap.tensor,
                            offset=g8ap.offset + j * H + n0,
                            ap=[g8ap.ap[0], [COLS * H, 2], [1, w]],
                        )
                        nc.tensor.matmul(
                            out=ps[:, n0:n1], lhsT=lhs_pair,
                            rhs=rhs_pair, start=True, stop=False,
                            perf_mode=mybir.MatmulPerfMode.DoubleRow)
                        nc.tensor.matmul(out=ps[:, n0:n1], lhsT=id8ap,
                                         rhs=cms8[:, j, n0:n1], start=False,
                                         stop=False)
                        nc.tensor.matmul(out=ps[:, n0:n1], lhsT=id16[:],
                                         rhs=posi16[:, j, n0:n1], start=False,
                                         stop=False)
                    # v2t linear feature into col 767: K=128, rhs column j is
                    # zero except rows 8j..8j+7 (no data deps - run early)
                    nc.tensor.matmul(out=ps[:, H1:H], lhsT=tv8[:],
                                     rhs=rhsv[:, j:j + 1], start=False,
                                     stop=False)
                    # merged sin stream covers cols 0..766; closes both groups
                    for n0, n1 in ((0, 512), (512, H1)):
                        nc.tensor.matmul(out=ps[:, n0:n1], lhsT=id16[:],
                                         rhs=Tg[:, c, n0:n1],
                                         start=False, stop=True)

                    nc.vector.bn_stats(out=st[:, c, 0, :], in_=ps[:, 0:384])
                    nc.vector.bn_stats(out=st[:, c, 1, :], in_=ps[:, 384:H])
                    nc.vector.bn_aggr(out=mv[:, c, :], in_=st[:, c, :, :])

                # rstd = 1/sqrt(var + eps); negmur = -mean * rstd
                mvap = mv[:]
                var_ap = bass.AP(tensor=mvap.tensor,
                                 offset=mvap.offset + 2 * c0 + 1,
                                 ap=[mvap.ap[0], [2, 2]])
                mean_ap = bass.AP(tensor=mvap.tensor,
                                  offset=mvap.offset + 2 * c0,
                                  ap=[mvap.ap[0], [2, 2]])
                nc.scalar.activation(out=rstd[:, c0:c0 + 2], in_=var_ap,
                                     func=Act.Sqrt, bias=eps_t[:], scale=1.0)
                nc.vector.reciprocal(out=rstd[:, c0:c0 + 2],
                                     in_=rstd[:, c0:c0 + 2])
                nc.vector.scalar_tensor_tensor(out=negmur[:, c0:c0 + 2],
                                               in0=mean_ap, scalar=-1.0,
                                               in1=rstd[:, c0:c0 + 2],
                                               op0=Alu.mult, op1=Alu.mult)
                for ci, c in enumerate((c0, c0 + 1)):
                    nc.scalar.activation(out=OUT[:, c, :], in_=pspair[ci][:, :],
                                         func=Act.Identity,
                                         scale=rstd[:, c:c + 1],
                                         bias=negmur[:, c:c + 1])
            if use_gamma_beta:
                def mid_bcast(ap2, reps):
                    return bass.AP(tensor=ap2.tensor, offset=ap2.offset,
                                   ap=[ap2.ap[0], [0, reps], ap2.ap[1]])
                nc.vector.tensor_tensor(out=OUT[:, :, :], in0=OUT[:, :, :],
                                        in1=mid_bcast(gamma_t[:], C),
                                        op=Alu.mult)
                nc.vector.tensor_tensor(out=OUT[:, :, :], in0=OUT[:, :, :],
                                        in1=mid_bcast(beta_t[:], C),
                                        op=Alu.add)

            nc.sync.dma_start(out=out_v[:, w0c:w0c + C, :], in_=OUT[:, :, :])

    nc.compile()
    return nc


def _get_nc(use_gamma_beta: bool):
    key = ("nc", use_gamma_beta)
    if key not in _cache:
        _cache[key] = _build(use_gamma_beta)
    return _cache[key]


def _f32a(x):
    return np.ascontiguousarray(np.asarray(x), dtype=np.float32)


def _i32a(x):
    return np.ascontiguousarray(np.asarray(x), dtype=np.int32)


def _pack_idx16(ids_row):
    # ids_row [S] -> [128, 128] int16: idx i at [i % 16, i // 16], the 16-row
    # wrap replicated to the 8 gpsimd core groups.
    arr = np.zeros((16, P), dtype=np.int16)
    arr[:, :] = ids_row.reshape(P, 16).T.astype(np.int16)
    return np.tile(arr, (8, 1))


def _split_bf16(x):
    import ml_dtypes
    hi = x.astype(ml_dtypes.bfloat16)
    lo = (x - hi.astype(np.float32)).astype(ml_dtypes.bfloat16)
    return hi, lo


_last_use_gb = False


def _make_in_maps(inputs, use_gb):
    import ml_dtypes
    bf16np = ml_dtypes.bfloat16
    fp8np = ml_dtypes.float8_e4m3

    word_ids = _i32a(inputs["word_ids"]).reshape(B, S)
    modal_ids = _i32a(inputs["modalities_ids"]).reshape(B, S)
    seg_ids = _i32a(inputs["seg_ids"]).reshape(B, S)
    npi_ids = _i32a(inputs["npi_ids"]).reshape(B, S)
    posi_ids = _i32a(inputs["posi_ids"]).reshape(B, S)
    age_tau = _f32a(inputs["age_tau"]).reshape(B, S)
    delay_tau = _f32a(inputs["delays_tau"]).reshape(B, S)
    cms_ids = modal_ids + MODAL_V * seg_ids

    modal_tbl = _f32a(inputs["modalities_table"]).reshape(MODAL_V, H)
    seg_tbl = _f32a(inputs["seg_table"]).reshape(SEG_V, H)
    # cms row (m + 16*s) = modal[m] + seg[s]
    cms_tbl = (modal_tbl[:, None, :] + seg_tbl[None, :, :]).reshape(
        MODAL_V * SEG_V, H)

    w_age = _f32a(inputs["age_w"]).reshape(H1)
    b_age = _f32a(inputs["age_b"]).reshape(H1)
    w0_age = float(_f32a(inputs["age_w0"]).reshape(1)[0])
    b0_age = float(_f32a(inputs["age_b0"]).reshape(1)[0])
    w_del = _f32a(inputs["delay_w"]).reshape(H1)
    b_del = _f32a(inputs["delay_b"]).reshape(H1)
    w0_del = float(_f32a(inputs["delay_w0"]).reshape(1)[0])
    b0_del = float(_f32a(inputs["delay_b0"]).reshape(1)[0])

    b0s = np.float32(b0_age + b0_del)
    w0ah, w0al = _split_bf16(np.float32(w0_age).reshape(1))
    w0dh, w0dl = _split_bf16(np.float32(w0_del).reshape(1))
    b0h, b0l = _split_bf16(b0s.reshape(1))
    rhs8_vec = np.stack([w0ah[0], w0al[0], w0ah[0], w0dh[0], w0dl[0], w0dh[0],
                         b0h[0], b0l[0]])
    # rhsv[:, j] is zero except rows 8j..8j+7 = rhs8_vec (pairs with tv8 rows)
    rhsv = np.zeros((P, COLS), dtype=bf16np)
    for j in range(COLS):
        rhsv[8 * j:8 * j + 8, j] = rhs8_vec

    shared = {
        "word_table": np.ascontiguousarray(
            _f32a(inputs["word_table"]).reshape(VOCAB, H).astype(fp8np)),
        "npi_table": np.ascontiguousarray(
            _f32a(inputs["npi_table"]).reshape(NPI_V, H).astype(fp8np)),
        "cms_table": np.ascontiguousarray(cms_tbl.astype(fp8np)),
        "posi_table": np.ascontiguousarray(
            _f32a(inputs["posi_table"]).reshape(MAX_POS, H).astype(bf16np)),
        "wa": np.ascontiguousarray(w_age * INV_2PI),
        "ba": np.ascontiguousarray(b_age * INV_2PI + KSHIFT),
        "wd": np.ascontiguousarray(w_del * INV_2PI),
        "bd": np.ascontiguousarray(b_del * INV_2PI + KSHIFT),
        "rhsv": np.ascontiguousarray(rhsv),
    }
    if use_gb:
        shared["ln_gamma"] = np.ascontiguousarray(
            _f32a(inputs["ln_gamma"]).reshape(H).astype(bf16np))
        shared["ln_beta"] = np.ascontiguousarray(
            _f32a(inputs["ln_beta"]).reshape(H).astype(bf16np))

    in_maps = []
    for i in range(B):
        # tv8: per col j (8 rows): [tau_age hi, hi, lo, tau_del hi, hi, lo, 1, 1]
        ta = age_tau[i].reshape(COLS, P)     # [j, p]
        td = delay_tau[i].reshape(COLS, P)
        tah, tal = _split_bf16(ta)
        tdh, tdl = _split_bf16(td)
        tv8 = np.zeros((P, P), dtype=bf16np)
        ones = np.ones(P, dtype=bf16np)
        for j in range(COLS):
            tv8[8 * j + 0] = tah[j]
            tv8[8 * j + 1] = tah[j]
            tv8[8 * j + 2] = tal[j]
            tv8[8 * j + 3] = tdh[j]
            tv8[8 * j + 4] = tdh[j]
            tv8[8 * j + 5] = tdl[j]
            tv8[8 * j + 6] = ones
            tv8[8 * j + 7] = ones
        m = dict(shared)
        m.update(
            word_idx16=_pack_idx16(word_ids[i]),
            npi_idx16=_pack_idx16(npi_ids[i]),
            cms_idx16=_pack_idx16(cms_ids[i]),
            posi_idx16=_pack_idx16(posi_ids[i]),
            age_tau=age_tau[i],
            delays_tau=delay_tau[i],
            tv8=np.ascontiguousarray(tv8),
        )
        in_maps.append(m)
    return in_maps


def kernel(**inputs) -> np.ndarray:
    global _last_use_gb
    from concourse.bass_utils import run_bass_kernel_spmd

    gamma = _f32a(inputs["ln_gamma"])
    beta = _f32a(inputs["ln_beta"])
    use_gb = not (np.all(gamma == 1.0) and np.all(beta == 0.0))
    _last_use_gb = use_gb
    nc = _get_nc(use_gb)
    in_maps = _make_in_maps(inputs, use_gb)
    core_ids = list(range(B))
    res = run_bass_kernel_spmd(nc, in_maps, core_ids)
    out = np.stack(
        [np.asarray(res.results[i]["out"]).astype(np.float32) for i in core_ids],
        axis=0)
    return out
